# revision 19
# baseline (speedup 1.0000x reference)
"""Trainium2 Bass kernel for the CWICDense (conditional stripe matmul) module.

Problem (hardcoded shapes):
  x          [2, 512, 4096] f32    tokens T=1024, features I=4096
  W_kernel   [4096, 4096]   f32    viewed as [I, N=32 stripes, Q=128]
  thresholds [4096, 32]     f32
  mu         [4096]         f32    (structurally zero in this module)
  out_mu     [4096]         f32
  where      [2, 512]       bool   (unused by the reference computation)

  y[t, n*Q+q] = sum_i x_off[t,i] * (|x_off[t,i]| >= thresholds[i,n]) * W[i, n*Q+q]
                + out_mu[n*Q+q]

Sharding across 8 NeuronCores: 8-way tensor parallel over stripes (4 stripes
= 512 out cols per core); every core sees all 1024 tokens. The host passes
x pre-transposed (pure layout prep) so the device does no transposes at all,
and each core returns its y^T block which the host transposes back.

Per-core device algorithm:
  - x^T k-tiles [128 features, 1024 tokens] DMA'd contiguously (resident).
  - per (stripe n, k-tile): one custom DVE instruction computes
      z = select((x >= t_n) | (x <= -t_n), x, 0)
    which equals x * (|x| >= t_n) exactly in fp32 (single input stream).
  - PE matmul (float32r, N=512 moving) accumulating y^T[n-block] in PSUM
    over 32 k-tiles: acc += W[k,n].T @ z, two token-halves per stripe.
  - epilogue: ACT adds out_mu (per-partition bias in y^T layout), DMA out.
"""

import sys

if "/opt/trn_rl_repo" not in sys.path:
    sys.path.insert(0, "/opt/trn_rl_repo")

import numpy as np

import concourse.bass as bass
import concourse.mybir as mybir
import concourse.tile as tile
from concourse import bacc, bass_utils
from concourse import dve_ops as _dve_ops
from concourse.dve_spec import Spec, Src0, C0, C1, Zero, select, lower
from concourse.dve_spec import _has_src1
from concourse.dve_table_gen import dve_ver_for
from concourse.dve_uop import DveOpSpec

# ---- problem constants -------------------------------------------------
B, S, I, N, Q = 2, 512, 4096, 32, 128
T = B * S                 # 1024 tokens
OUT = N * Q               # 4096
NCORES = 8
NS = N // NCORES          # 4 stripes per core
OUT_C = NS * Q            # 512 out cols per core
KT = I // 128             # 32 contraction tiles
P = 128
HF = 2                    # token halves per matmul group (1024 -> 2 x 512)
TH = T // HF              # 512

_CACHE = {}


def _register_gate_op():
    """Register the fused CWIC gate as a custom DVE op:
    out = select((in0 >= s0) | (in0 <= s1), in0, 0) — call with s0 = t,
    s1 = -t to get x * (|x| >= t) with exact fp32 compares."""
    name = "CWIC_GATE_ANT"
    if name in _dve_ops._SUB_OPCODE_FOR_NAME:
        return next(op for op in _dve_ops.OPS if op.name == name)
    spec = Spec(
        body=select((Src0 >= C0) | (Src0 <= C1), Src0, Zero),
        reference=lambda in0, in1, s0, s1, imm2: np.where(
            (in0 >= s0) | (in0 <= s1), in0, 0.0
        ).astype(np.float32),
    )
    row = max(_dve_ops._SUB_OPCODE_FOR_NAME.values()) + 1
    assert row < 0x20
    _dve_ops._SUB_OPCODE_FOR_NAME[name] = row
    shas = {}
    for ver in ("v3",):
        tmp = DveOpSpec(
            name=name, opcode=row, uops=lower(spec, ver=ver),
            rd1_en=_has_src1(spec),
        )
        shas[ver] = tmp.sha(ver)
    op = _dve_ops.DveOp(name, spec, subdim=False, uops_sha=shas)
    _dve_ops.OPS.append(op)
    _dve_ops.CUSTOM_DVE_SPECS[name] = spec
    return op


def _build():
    f32 = mybir.dt.float32
    f32r = mybir.dt.float32r
    gate_op = _register_gate_op()
    nc = bacc.Bacc("TRN2", target_bir_lowering=False, debug=False)

    xT_d = nc.dram_tensor("xT", [I, T], f32, kind="ExternalInput").ap()
    w_d = nc.dram_tensor("w", [I, OUT_C], f32, kind="ExternalInput").ap()
    # thr holds [t | -t]: cols 0..NS-1 are thresholds, NS..2*NS-1 negated
    thr_d = nc.dram_tensor("thr", [I, 2 * NS], f32, kind="ExternalInput").ap()
    mu_d = nc.dram_tensor("mu", [P, NS], f32, kind="ExternalInput").ap()
    yT_d = nc.dram_tensor("yT", [OUT_C, T], f32, kind="ExternalOutput").ap()

    # w_v[p, k, c] = w[k*128+p, c]
    w_v = w_d.rearrange("(k p) c -> p k c", p=P)

    with tile.TileContext(nc) as tc:
        with (
            tc.tile_pool(name="const", bufs=1) as constp,
            tc.tile_pool(name="xT", bufs=KT) as xTp,
            tc.tile_pool(name="thr", bufs=KT) as thrp,
            tc.tile_pool(name="w", bufs=8) as wp,
            tc.tile_pool(name="z", bufs=6) as zp,
            tc.tile_pool(name="yT", bufs=2) as yTp,
            tc.tile_pool(name="acc", bufs=4, space="PSUM") as accp,
            tc.tile_pool(name="warm", bufs=1, space="PSUM") as warmp,
        ):
            mu_sb = constp.tile([P, NS], f32, tag="mu")
            nc.sync.dma_start(mu_sb[:], mu_d)

            # interleave per-k threshold + x loads so z(n=0, k=0) can start
            # as soon as the first pair lands (per-tile dep granularity)
            xT = []
            thrT = []

            def load_pair(k):
                tk = thrp.tile([P, 2 * NS], f32, tag="thr", name=f"thr{k}")
                nc.sync.dma_start(tk[:], thr_d[k * P:(k + 1) * P, :])
                xk = xTp.tile([P, T], f32, tag="xT", name=f"xk{k}")
                nc.sync.dma_start(xk[:], xT_d[k * P:(k + 1) * P, :])
                xT.append(xk)
                thrT.append(tk)

            for k in range(5):
                load_pair(k)

            # HAM warm-up: a short burst of throwaway matmuls keyed on a DMA
            # that lands mid x-stream, so the PE clock is at 2.4 GHz just as
            # real matmuls arrive (a cold PE runs at 1.2 GHz).
            warmsrc = constp.tile([P, TH], f32r, tag="warmsrc")
            nc.sync.dma_start(warmsrc[:], xT_d[P:2 * P, 0:TH].bitcast(f32r))
            warm = warmp.tile([P, TH], f32, tag="warm")
            for _ in range(6):
                nc.tensor.matmul(
                    warm[:],
                    warmsrc[:, 0:P],
                    warmsrc[:],
                    start=True,
                    stop=True,
                )

            for k in range(5, KT):
                load_pair(k)

            KC = 8  # k-tiles per W chunk DMA
            for n in range(NS):
                # W streams in 8-k-tile chunks through the (otherwise idle)
                # scalar engine's DMA queue: they don't serialize behind the
                # x loads, and the first matmul only waits on a 512KB chunk.
                # wc[p, kk*Q+q] = w[(r*KC+kk)*128+p, n*Q+q]
                wcs = []
                for r in range(KT // KC):
                    wc = wp.tile([P, KC * Q], f32r, tag="w", name=f"wc{n}_{r}")
                    nc.scalar.dma_start(
                        wc[:].rearrange("p (k q) -> p k q", q=Q),
                        w_v[:, r * KC:(r + 1) * KC,
                            n * Q:(n + 1) * Q].bitcast(f32r),
                    )
                    wcs.append(wc)
                accs = [
                    accp.tile([P, TH], f32, tag="acc", name=f"acc{h}")
                    for h in range(HF)
                ]
                for k in range(KT):
                    zt = zp.tile([P, T], f32r, tag="z")
                    nc.vector._custom_dve(
                        gate_op,
                        out=zt[:],
                        in0=xT[k][:],
                        s0=thrT[k][:, n:n + 1],
                        s1=thrT[k][:, NS + n:NS + n + 1],
                    )
                    for h in range(HF):
                        nc.tensor.matmul(
                            accs[h][:],
                            wcs[k // KC][:, (k % KC) * Q:(k % KC + 1) * Q],
                            zt[:, h * TH:(h + 1) * TH],
                            start=(k == 0),
                            stop=(k == KT - 1),
                        )
                # epilogue: + out_mu (per-partition in y^T layout), DMA out
                for h in range(HF):
                    yt = yTp.tile([P, TH], f32, tag="yT")
                    nc.scalar.activation(
                        yt[:], accs[h][:],
                        mybir.ActivationFunctionType.Identity,
                        bias=mu_sb[:, n:n + 1],
                    )
                    nc.sync.dma_start(
                        yT_d[n * P:(n + 1) * P, h * TH:(h + 1) * TH], yt[:]
                    )
    nc.compile()
    return nc


def _get_nc():
    if "nc" not in _CACHE:
        _CACHE["nc"] = _build()
    return _CACHE["nc"]


def _make_in_maps(x, W_kernel, thresholds, mu, out_mu):
    xf = np.asarray(x, dtype=np.float32).reshape(T, I)
    xf = xf - np.asarray(mu, dtype=np.float32)[None, :]
    xT = np.ascontiguousarray(xf.T)
    W = np.asarray(W_kernel, np.float32)
    thr = np.asarray(thresholds, np.float32)
    omu = np.asarray(out_mu, np.float32)
    in_maps = []
    for g in range(NCORES):
        thr_c = thr[:, g * NS:(g + 1) * NS]
        in_maps.append({
            "xT": xT,
            "w": np.ascontiguousarray(W[:, g * OUT_C:(g + 1) * OUT_C]),
            "thr": np.ascontiguousarray(
                np.concatenate([thr_c, -thr_c], axis=1)
            ),
            "mu": np.ascontiguousarray(
                omu[g * OUT_C:(g + 1) * OUT_C].reshape(NS, P).T
            ),
        })
    return in_maps


def _assemble(results):
    yT = np.concatenate([results[g]["yT"] for g in range(NCORES)], axis=0)
    return np.ascontiguousarray(yT.T).reshape(B, S, OUT)


def run(inputs, **spmd_kwargs):
    """Run on hardware; returns (y, BassKernelResults)."""
    nc = _get_nc()
    in_maps = _make_in_maps(
        inputs["x"], inputs["W_kernel"], inputs["thresholds"],
        inputs["mu"], inputs["out_mu"],
    )
    res = bass_utils.run_bass_kernel_spmd(
        nc, in_maps, core_ids=list(range(NCORES)), **spmd_kwargs
    )
    return _assemble(res.results), res


def kernel(x, W_kernel, thresholds, mu, out_mu, where):
    y, _ = run({
        "x": x, "W_kernel": W_kernel, "thresholds": thresholds,
        "mu": mu, "out_mu": out_mu, "where": where,
    })
    return y


# revision 21
# speedup vs baseline: 1.1524x; 1.1524x over previous
"""Trainium2 Bass kernel for the CWICDense (conditional stripe matmul) module.

Problem (hardcoded shapes):
  x          [2, 512, 4096] f32    tokens T=1024, features I=4096
  W_kernel   [4096, 4096]   f32    viewed as [I, N=32 stripes, Q=128]
  thresholds [4096, 32]     f32
  mu         [4096]         f32    (structurally zero in this module)
  out_mu     [4096]         f32
  where      [2, 512]       bool   (unused by the reference computation)

  y[t, n*Q+q] = sum_i x_off[t,i] * (|x_off[t,i]| >= thresholds[i,n]) * W[i, n*Q+q]
                + out_mu[n*Q+q]

Sharding across 8 NeuronCores: 8-way tensor parallel over stripes (4 stripes
= 512 out cols per core); every core sees all 1024 tokens. The host passes
x pre-transposed (pure layout prep) so the device does no transposes at all,
and each core returns its y^T block which the host transposes back.

Per-core device algorithm:
  - x^T k-tiles [128 features, 1024 tokens] DMA'd contiguously (resident).
  - per (stripe n, k-tile): one custom DVE instruction computes
      z = select((x >= t_n) | (x <= -t_n), x, 0)
    which equals x * (|x| >= t_n) exactly in fp32 (single input stream).
  - PE matmul (float32r, N=512 moving) accumulating y^T[n-block] in PSUM
    over 32 k-tiles: acc += W[k,n].T @ z, two token-halves per stripe.
  - epilogue: ACT adds out_mu (per-partition bias in y^T layout), DMA out.
"""

import sys

if "/opt/trn_rl_repo" not in sys.path:
    sys.path.insert(0, "/opt/trn_rl_repo")

import numpy as np

import concourse.bass as bass
import concourse.mybir as mybir
import concourse.tile as tile
from concourse import bacc, bass_utils
from concourse import dve_ops as _dve_ops
from concourse.dve_spec import Spec, Src0, C0, C1, Zero, select, lower
from concourse.dve_spec import _has_src1
from concourse.dve_table_gen import dve_ver_for
from concourse.dve_uop import DveOpSpec

# ---- problem constants -------------------------------------------------
B, S, I, N, Q = 2, 512, 4096, 32, 128
T = B * S                 # 1024 tokens
OUT = N * Q               # 4096
NCORES = 8
NS = N // NCORES          # 4 stripes per core
OUT_C = NS * Q            # 512 out cols per core
KT = I // 128             # 32 contraction tiles
P = 128
HF = 2                    # token halves per matmul group (1024 -> 2 x 512)
TH = T // HF              # 512

_CACHE = {}


def _register_gate_op():
    """Register the fused CWIC gate as a custom DVE op:
    out = select((in0 >= s0) | (in0 <= s1), in0, 0) — call with s0 = t,
    s1 = -t to get x * (|x| >= t) with exact fp32 compares."""
    name = "CWIC_GATE_ANT"
    if name in _dve_ops._SUB_OPCODE_FOR_NAME:
        return next(op for op in _dve_ops.OPS if op.name == name)
    spec = Spec(
        body=select((Src0 >= C0) | (Src0 <= C1), Src0, Zero),
        reference=lambda in0, in1, s0, s1, imm2: np.where(
            (in0 >= s0) | (in0 <= s1), in0, 0.0
        ).astype(np.float32),
    )
    row = max(_dve_ops._SUB_OPCODE_FOR_NAME.values()) + 1
    assert row < 0x20
    _dve_ops._SUB_OPCODE_FOR_NAME[name] = row
    shas = {}
    for ver in ("v3",):
        tmp = DveOpSpec(
            name=name, opcode=row, uops=lower(spec, ver=ver),
            rd1_en=_has_src1(spec),
        )
        shas[ver] = tmp.sha(ver)
    op = _dve_ops.DveOp(name, spec, subdim=False, uops_sha=shas)
    _dve_ops.OPS.append(op)
    _dve_ops.CUSTOM_DVE_SPECS[name] = spec
    return op


def _build():
    f32 = mybir.dt.float32
    f32r = mybir.dt.float32r
    gate_op = _register_gate_op()
    nc = bacc.Bacc("TRN2", target_bir_lowering=False, debug=False)

    xT_d = nc.dram_tensor("xT", [I, T], f32, kind="ExternalInput").ap()
    w_d = nc.dram_tensor("w", [I, OUT_C], f32, kind="ExternalInput").ap()
    # thr holds [t | -t]: cols 0..NS-1 are thresholds, NS..2*NS-1 negated
    thr_d = nc.dram_tensor("thr", [I, 2 * NS], f32, kind="ExternalInput").ap()
    mu_d = nc.dram_tensor("mu", [P, NS], f32, kind="ExternalInput").ap()
    yT_d = nc.dram_tensor("yT", [OUT_C, T], f32, kind="ExternalOutput").ap()

    # w_v[p, k, c] = w[k*128+p, c]
    w_v = w_d.rearrange("(k p) c -> p k c", p=P)

    with tile.TileContext(nc) as tc:
        with (
            tc.tile_pool(name="const", bufs=1) as constp,
            tc.tile_pool(name="xT", bufs=22) as xTp,
            tc.tile_pool(name="thr", bufs=KT) as thrp,
            tc.tile_pool(name="w", bufs=12) as wp,
            tc.tile_pool(name="z", bufs=8) as zp,
            tc.tile_pool(name="yT", bufs=2) as yTp,
            tc.tile_pool(name="acc", bufs=2 * NS, space="PSUM") as accp,
        ):
            mu_sb = constp.tile([P, NS], f32, tag="mu")
            nc.sync.dma_start(mu_sb[:], mu_d)

            # interleave per-k threshold + x loads so z(n=0, k=0) can start
            # as soon as the first pair lands (per-tile dep granularity)
            xT = []
            thrT = []

            def load_pair(k):
                tk = thrp.tile([P, 2 * NS], f32, tag="thr", name=f"thr{k}")
                nc.sync.dma_start(tk[:], thr_d[k * P:(k + 1) * P, :])
                xk = xTp.tile([P, T], f32, tag="xT", name=f"xk{k}")
                nc.sync.dma_start(xk[:], xT_d[k * P:(k + 1) * P, :])
                xT.append(xk)
                thrT.append(tk)

            for k in range(2):
                load_pair(k)

            # HAM warm-up: a short burst of throwaway matmuls keyed on a DMA
            # that lands early in the x stream, so the PE clock is at 2.4 GHz
            # as real matmuls arrive (a cold PE runs at 1.2 GHz). The burst
            # writes into an accumulator bank that the real k=0 matmul
            # (start=True) resets afterwards.
            warmsrc = constp.tile([P, TH], f32r, tag="warmsrc")
            nc.sync.dma_start(warmsrc[:], xT_d[P:2 * P, 0:TH].bitcast(f32r))

            for k in range(2, KT):
                load_pair(k)

            # all W chunks issued upfront on the scalar engine's DMA queue;
            # transfers stream in consumption (r-major) order.
            # wc[p, kk*Q+q] = w[(r*KC+kk)*128+p, n*Q+q]
            KC = 8  # k-tiles per W chunk DMA
            wcs = {}
            for r in range(KT // KC):
                for n in range(NS):
                    wc = wp.tile([P, KC * Q], f32r, tag="w", name=f"wc{n}_{r}")
                    nc.scalar.dma_start(
                        wc[:].rearrange("p (k q) -> p k q", q=Q),
                        w_v[:, r * KC:(r + 1) * KC,
                            n * Q:(n + 1) * Q].bitcast(f32r),
                    )
                    wcs[(n, r)] = wc

            accs = [
                accp.tile([P, TH], f32, tag="acc", name=f"acc{n}_{h}")
                for n in range(NS) for h in range(HF)
            ]
            for _ in range(6):
                nc.tensor.matmul(
                    accs[-1][:],
                    warmsrc[:, 0:P],
                    warmsrc[:],
                    start=True,
                    stop=True,
                )

            # k-outer: every x tile is consumed immediately by all 4 stripes,
            # so the x stream spreads over the whole kernel instead of
            # piling into the first stripe's window.
            for k in range(KT):
                for n in range(NS):
                    zt = zp.tile([P, T], f32r, tag="z")
                    nc.vector._custom_dve(
                        gate_op,
                        out=zt[:],
                        in0=xT[k][:],
                        s0=thrT[k][:, n:n + 1],
                        s1=thrT[k][:, NS + n:NS + n + 1],
                    )
                    for h in range(HF):
                        nc.tensor.matmul(
                            accs[n * HF + h][:],
                            wcs[(n, k // KC)][:, (k % KC) * Q:(k % KC + 1) * Q],
                            zt[:, h * TH:(h + 1) * TH],
                            start=(k == 0),
                            stop=(k == KT - 1),
                        )
            # epilogue: + out_mu (per-partition in y^T layout), DMA out
            for n in range(NS):
                for h in range(HF):
                    yt = yTp.tile([P, TH], f32, tag="yT")
                    nc.scalar.activation(
                        yt[:], accs[n * HF + h][:],
                        mybir.ActivationFunctionType.Identity,
                        bias=mu_sb[:, n:n + 1],
                    )
                    nc.sync.dma_start(
                        yT_d[n * P:(n + 1) * P, h * TH:(h + 1) * TH], yt[:]
                    )
    nc.compile()
    return nc


def _get_nc():
    if "nc" not in _CACHE:
        _CACHE["nc"] = _build()
    return _CACHE["nc"]


def _make_in_maps(x, W_kernel, thresholds, mu, out_mu):
    xf = np.asarray(x, dtype=np.float32).reshape(T, I)
    xf = xf - np.asarray(mu, dtype=np.float32)[None, :]
    xT = np.ascontiguousarray(xf.T)
    W = np.asarray(W_kernel, np.float32)
    thr = np.asarray(thresholds, np.float32)
    omu = np.asarray(out_mu, np.float32)
    in_maps = []
    for g in range(NCORES):
        thr_c = thr[:, g * NS:(g + 1) * NS]
        in_maps.append({
            "xT": xT,
            "w": np.ascontiguousarray(W[:, g * OUT_C:(g + 1) * OUT_C]),
            "thr": np.ascontiguousarray(
                np.concatenate([thr_c, -thr_c], axis=1)
            ),
            "mu": np.ascontiguousarray(
                omu[g * OUT_C:(g + 1) * OUT_C].reshape(NS, P).T
            ),
        })
    return in_maps


def _assemble(results):
    yT = np.concatenate([results[g]["yT"] for g in range(NCORES)], axis=0)
    return np.ascontiguousarray(yT.T).reshape(B, S, OUT)


def run(inputs, **spmd_kwargs):
    """Run on hardware; returns (y, BassKernelResults)."""
    nc = _get_nc()
    in_maps = _make_in_maps(
        inputs["x"], inputs["W_kernel"], inputs["thresholds"],
        inputs["mu"], inputs["out_mu"],
    )
    res = bass_utils.run_bass_kernel_spmd(
        nc, in_maps, core_ids=list(range(NCORES)), **spmd_kwargs
    )
    return _assemble(res.results), res


def kernel(x, W_kernel, thresholds, mu, out_mu, where):
    y, _ = run({
        "x": x, "W_kernel": W_kernel, "thresholds": thresholds,
        "mu": mu, "out_mu": out_mu, "where": where,
    })
    return y


# revision 22
# speedup vs baseline: 1.1714x; 1.0165x over previous
"""Trainium2 Bass kernel for the CWICDense (conditional stripe matmul) module.

Problem (hardcoded shapes):
  x          [2, 512, 4096] f32    tokens T=1024, features I=4096
  W_kernel   [4096, 4096]   f32    viewed as [I, N=32 stripes, Q=128]
  thresholds [4096, 32]     f32
  mu         [4096]         f32    (structurally zero in this module)
  out_mu     [4096]         f32
  where      [2, 512]       bool   (unused by the reference computation)

  y[t, n*Q+q] = sum_i x_off[t,i] * (|x_off[t,i]| >= thresholds[i,n]) * W[i, n*Q+q]
                + out_mu[n*Q+q]

Sharding across 8 NeuronCores: 8-way tensor parallel over stripes (4 stripes
= 512 out cols per core); every core sees all 1024 tokens. The host passes
x pre-transposed (pure layout prep) so the device does no transposes at all,
and each core returns its y^T block which the host transposes back.

Per-core device algorithm:
  - x^T k-tiles [128 features, 1024 tokens] DMA'd contiguously (resident).
  - per (stripe n, k-tile): one custom DVE instruction computes
      z = select((x >= t_n) | (x <= -t_n), x, 0)
    which equals x * (|x| >= t_n) exactly in fp32 (single input stream).
  - PE matmul (float32r, N=512 moving) accumulating y^T[n-block] in PSUM
    over 32 k-tiles: acc += W[k,n].T @ z, two token-halves per stripe.
  - epilogue: ACT adds out_mu (per-partition bias in y^T layout), DMA out.
"""

import sys

if "/opt/trn_rl_repo" not in sys.path:
    sys.path.insert(0, "/opt/trn_rl_repo")

import numpy as np

import concourse.bass as bass
import concourse.mybir as mybir
import concourse.tile as tile
from concourse import bacc, bass_utils
from concourse import dve_ops as _dve_ops
from concourse.dve_spec import Spec, Src0, C0, C1, Zero, select, lower
from concourse.dve_spec import _has_src1
from concourse.dve_table_gen import dve_ver_for
from concourse.dve_uop import DveOpSpec

# ---- problem constants -------------------------------------------------
B, S, I, N, Q = 2, 512, 4096, 32, 128
T = B * S                 # 1024 tokens
OUT = N * Q               # 4096
NCORES = 8
NS = N // NCORES          # 4 stripes per core
OUT_C = NS * Q            # 512 out cols per core
KT = I // 128             # 32 contraction tiles
P = 128
HF = 2                    # token halves per matmul group (1024 -> 2 x 512)
TH = T // HF              # 512

_CACHE = {}


def _register_gate_op():
    """Register the fused CWIC gate as a custom DVE op:
    out = select((in0 >= s0) | (in0 <= s1), in0, 0) — call with s0 = t,
    s1 = -t to get x * (|x| >= t) with exact fp32 compares."""
    name = "CWIC_GATE_ANT"
    if name in _dve_ops._SUB_OPCODE_FOR_NAME:
        return next(op for op in _dve_ops.OPS if op.name == name)
    spec = Spec(
        body=select((Src0 >= C0) | (Src0 <= C1), Src0, Zero),
        reference=lambda in0, in1, s0, s1, imm2: np.where(
            (in0 >= s0) | (in0 <= s1), in0, 0.0
        ).astype(np.float32),
    )
    row = max(_dve_ops._SUB_OPCODE_FOR_NAME.values()) + 1
    assert row < 0x20
    _dve_ops._SUB_OPCODE_FOR_NAME[name] = row
    shas = {}
    for ver in ("v3",):
        tmp = DveOpSpec(
            name=name, opcode=row, uops=lower(spec, ver=ver),
            rd1_en=_has_src1(spec),
        )
        shas[ver] = tmp.sha(ver)
    op = _dve_ops.DveOp(name, spec, subdim=False, uops_sha=shas)
    _dve_ops.OPS.append(op)
    _dve_ops.CUSTOM_DVE_SPECS[name] = spec
    return op


def _build():
    f32 = mybir.dt.float32
    f32r = mybir.dt.float32r
    gate_op = _register_gate_op()
    nc = bacc.Bacc("TRN2", target_bir_lowering=False, debug=False)

    xT_d = nc.dram_tensor("xT", [I, T], f32, kind="ExternalInput").ap()
    w_d = nc.dram_tensor("w", [I, OUT_C], f32, kind="ExternalInput").ap()
    # thr holds [t | -t]: cols 0..NS-1 are thresholds, NS..2*NS-1 negated
    thr_d = nc.dram_tensor("thr", [I, 2 * NS], f32, kind="ExternalInput").ap()
    mu_d = nc.dram_tensor("mu", [P, NS], f32, kind="ExternalInput").ap()
    yT_d = nc.dram_tensor("yT", [OUT_C, T], f32, kind="ExternalOutput").ap()

    # w_v[p, k, c] = w[k*128+p, c]
    w_v = w_d.rearrange("(k p) c -> p k c", p=P)

    with tile.TileContext(nc) as tc:
        with (
            tc.tile_pool(name="const", bufs=1) as constp,
            tc.tile_pool(name="xT", bufs=22) as xTp,
            tc.tile_pool(name="thr", bufs=KT) as thrp,
            tc.tile_pool(name="w", bufs=12) as wp,
            tc.tile_pool(name="z", bufs=10) as zp,
            tc.tile_pool(name="yT", bufs=6) as yTp,
            tc.tile_pool(name="acc", bufs=2 * NS, space="PSUM") as accp,
        ):
            mu_sb = constp.tile([P, NS], f32, tag="mu")
            nc.sync.dma_start(mu_sb[:], mu_d)

            # interleave per-k threshold + x loads so z(n=0, k=0) can start
            # as soon as the first pair lands (per-tile dep granularity)
            xT = []
            thrT = []

            def load_pair(k):
                tk = thrp.tile([P, 2 * NS], f32, tag="thr", name=f"thr{k}")
                nc.sync.dma_start(tk[:], thr_d[k * P:(k + 1) * P, :])
                xk = xTp.tile([P, T], f32, tag="xT", name=f"xk{k}")
                nc.sync.dma_start(xk[:], xT_d[k * P:(k + 1) * P, :])
                xT.append(xk)
                thrT.append(tk)

            for k in range(2):
                load_pair(k)

            # HAM warm-up: a short burst of throwaway matmuls keyed on a DMA
            # that lands early in the x stream, so the PE clock is at 2.4 GHz
            # as real matmuls arrive (a cold PE runs at 1.2 GHz). The burst
            # writes into an accumulator bank that the real k=0 matmul
            # (start=True) resets afterwards.
            warmsrc = constp.tile([P, TH], f32r, tag="warmsrc")
            nc.sync.dma_start(warmsrc[:], xT_d[P:2 * P, 0:TH].bitcast(f32r))

            for k in range(2, KT):
                load_pair(k)

            # all W chunks issued upfront on the scalar engine's DMA queue;
            # transfers stream in consumption (r-major) order.
            # wc[p, kk*Q+q] = w[(r*KC+kk)*128+p, n*Q+q]
            KC = 8  # k-tiles per W chunk DMA
            wcs = {}
            for r in range(KT // KC):
                for n in range(NS):
                    wc = wp.tile([P, KC * Q], f32r, tag="w", name=f"wc{n}_{r}")
                    nc.scalar.dma_start(
                        wc[:].rearrange("p (k q) -> p k q", q=Q),
                        w_v[:, r * KC:(r + 1) * KC,
                            n * Q:(n + 1) * Q].bitcast(f32r),
                    )
                    wcs[(n, r)] = wc

            accs = [
                accp.tile([P, TH], f32, tag="acc", name=f"acc{n}_{h}")
                for n in range(NS) for h in range(HF)
            ]
            for _ in range(6):
                nc.tensor.matmul(
                    accs[-1][:],
                    warmsrc[:, 0:P],
                    warmsrc[:],
                    start=True,
                    stop=True,
                )

            # k-outer: every x tile is consumed immediately by all 4 stripes,
            # so the x stream spreads over the whole kernel instead of
            # piling into the first stripe's window.
            for k in range(KT):
                for n in range(NS):
                    zt = zp.tile([P, T], f32r, tag="z")
                    nc.vector._custom_dve(
                        gate_op,
                        out=zt[:],
                        in0=xT[k][:],
                        s0=thrT[k][:, n:n + 1],
                        s1=thrT[k][:, NS + n:NS + n + 1],
                    )
                    for h in range(HF):
                        nc.tensor.matmul(
                            accs[n * HF + h][:],
                            wcs[(n, k // KC)][:, (k % KC) * Q:(k % KC + 1) * Q],
                            zt[:, h * TH:(h + 1) * TH],
                            start=(k == 0),
                            stop=(k == KT - 1),
                        )
            # epilogue: + out_mu (per-partition in y^T layout), DMA out
            for n in range(NS):
                for h in range(HF):
                    yt = yTp.tile([P, TH], f32, tag="yT")
                    nc.scalar.activation(
                        yt[:], accs[n * HF + h][:],
                        mybir.ActivationFunctionType.Identity,
                        bias=mu_sb[:, n:n + 1],
                    )
                    nc.sync.dma_start(
                        yT_d[n * P:(n + 1) * P, h * TH:(h + 1) * TH], yt[:]
                    )
    nc.compile()
    return nc


def _get_nc():
    if "nc" not in _CACHE:
        _CACHE["nc"] = _build()
    return _CACHE["nc"]


def _make_in_maps(x, W_kernel, thresholds, mu, out_mu):
    xf = np.asarray(x, dtype=np.float32).reshape(T, I)
    xf = xf - np.asarray(mu, dtype=np.float32)[None, :]
    xT = np.ascontiguousarray(xf.T)
    W = np.asarray(W_kernel, np.float32)
    thr = np.asarray(thresholds, np.float32)
    omu = np.asarray(out_mu, np.float32)
    in_maps = []
    for g in range(NCORES):
        thr_c = thr[:, g * NS:(g + 1) * NS]
        in_maps.append({
            "xT": xT,
            "w": np.ascontiguousarray(W[:, g * OUT_C:(g + 1) * OUT_C]),
            "thr": np.ascontiguousarray(
                np.concatenate([thr_c, -thr_c], axis=1)
            ),
            "mu": np.ascontiguousarray(
                omu[g * OUT_C:(g + 1) * OUT_C].reshape(NS, P).T
            ),
        })
    return in_maps


def _assemble(results):
    yT = np.concatenate([results[g]["yT"] for g in range(NCORES)], axis=0)
    return np.ascontiguousarray(yT.T).reshape(B, S, OUT)


def run(inputs, **spmd_kwargs):
    """Run on hardware; returns (y, BassKernelResults)."""
    nc = _get_nc()
    in_maps = _make_in_maps(
        inputs["x"], inputs["W_kernel"], inputs["thresholds"],
        inputs["mu"], inputs["out_mu"],
    )
    res = bass_utils.run_bass_kernel_spmd(
        nc, in_maps, core_ids=list(range(NCORES)), **spmd_kwargs
    )
    return _assemble(res.results), res


def kernel(x, W_kernel, thresholds, mu, out_mu, where):
    y, _ = run({
        "x": x, "W_kernel": W_kernel, "thresholds": thresholds,
        "mu": mu, "out_mu": out_mu, "where": where,
    })
    return y


# revision 25
# speedup vs baseline: 1.2021x; 1.0263x over previous
"""Trainium2 Bass kernel for the CWICDense (conditional stripe matmul) module.

Problem (hardcoded shapes):
  x          [2, 512, 4096] f32    tokens T=1024, features I=4096
  W_kernel   [4096, 4096]   f32    viewed as [I, N=32 stripes, Q=128]
  thresholds [4096, 32]     f32
  mu         [4096]         f32    (structurally zero in this module)
  out_mu     [4096]         f32
  where      [2, 512]       bool   (unused by the reference computation)

  y[t, n*Q+q] = sum_i x_off[t,i] * (|x_off[t,i]| >= thresholds[i,n]) * W[i, n*Q+q]
                + out_mu[n*Q+q]

Sharding across 8 NeuronCores: 8-way tensor parallel over stripes (4 stripes
= 512 out cols per core); every core sees all 1024 tokens. The host passes
x pre-transposed (pure layout prep) so the device does no transposes at all,
and each core returns its y^T block which the host transposes back.

Per-core device algorithm:
  - x^T k-tiles [128 features, 1024 tokens] DMA'd contiguously (resident).
  - per (stripe n, k-tile): one custom DVE instruction computes
      z = select((x >= t_n) | (x <= -t_n), x, 0)
    which equals x * (|x| >= t_n) exactly in fp32 (single input stream).
  - PE matmul (float32r, N=512 moving) accumulating y^T[n-block] in PSUM
    over 32 k-tiles: acc += W[k,n].T @ z, two token-halves per stripe.
  - epilogue: ACT adds out_mu (per-partition bias in y^T layout), DMA out.
"""

import sys

if "/opt/trn_rl_repo" not in sys.path:
    sys.path.insert(0, "/opt/trn_rl_repo")

import numpy as np

import concourse.mybir as mybir
import concourse.tile as tile
from concourse import bacc, bass_utils
from concourse import dve_ops as _dve_ops
from concourse.dve_spec import Spec, Src0, C0, C1, Zero, select, lower
from concourse.dve_spec import _has_src1
from concourse.dve_uop import DveOpSpec

# ---- problem constants -------------------------------------------------
B, S, I, N, Q = 2, 512, 4096, 32, 128
T = B * S                 # 1024 tokens
OUT = N * Q               # 4096
NCORES = 8
NS = N // NCORES          # 4 stripes per core
OUT_C = NS * Q            # 512 out cols per core
KT = I // 128             # 32 contraction tiles
P = 128
HF = 2                    # token halves per matmul group (1024 -> 2 x 512)
TH = T // HF              # 512

_CACHE = {}


def _register_gate_op():
    """Register the fused CWIC gate as a custom DVE op:
    out = select((in0 >= s0) | (in0 <= s1), in0, 0) — call with s0 = t,
    s1 = -t to get x * (|x| >= t) with exact fp32 compares."""
    name = "CWIC_GATE_ANT"
    if name in _dve_ops._SUB_OPCODE_FOR_NAME:
        return next(op for op in _dve_ops.OPS if op.name == name)
    spec = Spec(
        body=select((Src0 >= C0) | (Src0 <= C1), Src0, Zero),
        reference=lambda in0, in1, s0, s1, imm2: np.where(
            (in0 >= s0) | (in0 <= s1), in0, 0.0
        ).astype(np.float32),
    )
    row = max(_dve_ops._SUB_OPCODE_FOR_NAME.values()) + 1
    assert row < 0x20
    _dve_ops._SUB_OPCODE_FOR_NAME[name] = row
    shas = {}
    for ver in ("v3",):
        tmp = DveOpSpec(
            name=name, opcode=row, uops=lower(spec, ver=ver),
            rd1_en=_has_src1(spec),
        )
        shas[ver] = tmp.sha(ver)
    op = _dve_ops.DveOp(name, spec, subdim=False, uops_sha=shas)
    _dve_ops.OPS.append(op)
    _dve_ops.CUSTOM_DVE_SPECS[name] = spec
    return op


def _build():
    f32 = mybir.dt.float32
    f32r = mybir.dt.float32r
    gate_op = _register_gate_op()
    nc = bacc.Bacc("TRN2", target_bir_lowering=False, debug=False)

    xT_d = nc.dram_tensor("xT", [I, T], f32, kind="ExternalInput").ap()
    w_d = nc.dram_tensor("w", [I, OUT_C], f32, kind="ExternalInput").ap()
    # thr holds [t | -t]: cols 0..NS-1 are thresholds, NS..2*NS-1 negated
    thr_d = nc.dram_tensor("thr", [I, 2 * NS], f32, kind="ExternalInput").ap()
    mu_d = nc.dram_tensor("mu", [P, NS], f32, kind="ExternalInput").ap()
    yT_d = nc.dram_tensor("yT", [OUT_C, T], f32, kind="ExternalOutput").ap()

    # w_v[p, k, c] = w[k*128+p, c]
    w_v = w_d.rearrange("(k p) c -> p k c", p=P)

    with tile.TileContext(nc) as tc:
        with (
            tc.tile_pool(name="const", bufs=1) as constp,
            tc.tile_pool(name="xT", bufs=22) as xTp,
            tc.tile_pool(name="thr", bufs=KT) as thrp,
            tc.tile_pool(name="w", bufs=12) as wp,
            tc.tile_pool(name="z", bufs=10) as zp,
            tc.tile_pool(name="yT", bufs=6) as yTp,
            tc.tile_pool(name="acc", bufs=2 * NS, space="PSUM") as accp,
        ):
            # interleave per-k threshold + x loads so z(n=0, k=0) can start
            # as soon as the first pair lands (per-tile dep granularity)
            xT = []
            thrT = []

            def load_pair(k):
                tk = thrp.tile([P, 2 * NS], f32, tag="thr", name=f"thr{k}")
                nc.sync.dma_start(tk[:], thr_d[k * P:(k + 1) * P, :])
                xk = xTp.tile([P, T], f32, tag="xT", name=f"xk{k}")
                nc.sync.dma_start(xk[:], xT_d[k * P:(k + 1) * P, :])
                xT.append(xk)
                thrT.append(tk)

            for k in range(2):
                load_pair(k)

            # HAM warm-up: a short burst of throwaway matmuls keyed on a DMA
            # that lands early in the x stream, so the PE clock is at 2.4 GHz
            # as real matmuls arrive (a cold PE runs at 1.2 GHz). The burst
            # writes into an accumulator bank that the real k=0 matmul
            # (start=True) resets afterwards.
            warmsrc = constp.tile([P, TH], f32r, tag="warmsrc")
            nc.sync.dma_start(warmsrc[:], xT_d[P:2 * P, 0:TH].bitcast(f32r))

            for k in range(2, KT):
                load_pair(k)

            # all W chunks issued upfront on the scalar engine's DMA queue;
            # transfers stream in consumption (r-major) order.
            # wc[p, kk*Q+q] = w[(r*KC+kk)*128+p, n*Q+q]
            KC = 8  # k-tiles per W chunk DMA
            wcs = {}
            for r in range(KT // KC):
                for n in range(NS):
                    wc = wp.tile([P, KC * Q], f32r, tag="w", name=f"wc{n}_{r}")
                    nc.scalar.dma_start(
                        wc[:].rearrange("p (k q) -> p k q", q=Q),
                        w_v[:, r * KC:(r + 1) * KC,
                            n * Q:(n + 1) * Q].bitcast(f32r),
                    )
                    wcs[(n, r)] = wc

            # out_mu is only needed at the epilogue — keep it off the
            # critical sync-queue head
            mu_sb = constp.tile([P, NS], f32, tag="mu")
            nc.scalar.dma_start(mu_sb[:], mu_d)

            accs = [
                accp.tile([P, TH], f32, tag="acc", name=f"acc{n}_{h}")
                for n in range(NS) for h in range(HF)
            ]
            for _ in range(6):
                nc.tensor.matmul(
                    accs[-1][:],
                    warmsrc[:, 0:P],
                    warmsrc[:],
                    start=True,
                    stop=True,
                )

            # k-outer: every x tile is consumed immediately by all 4 stripes,
            # so the x stream spreads over the whole kernel instead of
            # piling into the first stripe's window.
            for k in range(KT):
                for n in range(NS):
                    zt = zp.tile([P, T], f32r, tag="z")
                    nc.vector._custom_dve(
                        gate_op,
                        out=zt[:],
                        in0=xT[k][:],
                        s0=thrT[k][:, n:n + 1],
                        s1=thrT[k][:, NS + n:NS + n + 1],
                    )
                    for h in range(HF):
                        nc.tensor.matmul(
                            accs[n * HF + h][:],
                            wcs[(n, k // KC)][:, (k % KC) * Q:(k % KC + 1) * Q],
                            zt[:, h * TH:(h + 1) * TH],
                            start=(k == 0),
                            stop=(k == KT - 1),
                        )
            # epilogue: + out_mu (per-partition in y^T layout), DMA out
            for n in range(NS):
                for h in range(HF):
                    yt = yTp.tile([P, TH], f32, tag="yT")
                    nc.scalar.activation(
                        yt[:], accs[n * HF + h][:],
                        mybir.ActivationFunctionType.Identity,
                        bias=mu_sb[:, n:n + 1],
                    )
                    nc.sync.dma_start(
                        yT_d[n * P:(n + 1) * P, h * TH:(h + 1) * TH], yt[:]
                    )
    nc.compile()
    return nc


def _get_nc():
    if "nc" not in _CACHE:
        _CACHE["nc"] = _build()
    return _CACHE["nc"]


def _make_in_maps(x, W_kernel, thresholds, mu, out_mu):
    xf = np.asarray(x, dtype=np.float32).reshape(T, I)
    xf = xf - np.asarray(mu, dtype=np.float32)[None, :]
    xT = np.ascontiguousarray(xf.T)
    W = np.asarray(W_kernel, np.float32)
    thr = np.asarray(thresholds, np.float32)
    omu = np.asarray(out_mu, np.float32)
    in_maps = []
    for g in range(NCORES):
        thr_c = thr[:, g * NS:(g + 1) * NS]
        in_maps.append({
            "xT": xT,
            "w": np.ascontiguousarray(W[:, g * OUT_C:(g + 1) * OUT_C]),
            "thr": np.ascontiguousarray(
                np.concatenate([thr_c, -thr_c], axis=1)
            ),
            "mu": np.ascontiguousarray(
                omu[g * OUT_C:(g + 1) * OUT_C].reshape(NS, P).T
            ),
        })
    return in_maps


def _assemble(results):
    yT = np.concatenate([results[g]["yT"] for g in range(NCORES)], axis=0)
    return np.ascontiguousarray(yT.T).reshape(B, S, OUT)


def run(inputs, **spmd_kwargs):
    """Run on hardware; returns (y, BassKernelResults)."""
    nc = _get_nc()
    in_maps = _make_in_maps(
        inputs["x"], inputs["W_kernel"], inputs["thresholds"],
        inputs["mu"], inputs["out_mu"],
    )
    res = bass_utils.run_bass_kernel_spmd(
        nc, in_maps, core_ids=list(range(NCORES)), **spmd_kwargs
    )
    return _assemble(res.results), res


def kernel(x, W_kernel, thresholds, mu, out_mu, where):
    y, _ = run({
        "x": x, "W_kernel": W_kernel, "thresholds": thresholds,
        "mu": mu, "out_mu": out_mu, "where": where,
    })
    return y


# revision 32
# speedup vs baseline: 1.2688x; 1.0555x over previous
"""Trainium2 Bass kernel for the CWICDense (conditional stripe matmul) module.

Problem (hardcoded shapes):
  x          [2, 512, 4096] f32    tokens T=1024, features I=4096
  W_kernel   [4096, 4096]   f32    viewed as [I, N=32 stripes, Q=128]
  thresholds [4096, 32]     f32
  mu         [4096]         f32    (structurally zero in this module)
  out_mu     [4096]         f32
  where      [2, 512]       bool   (unused by the reference computation)

  y[t, n*Q+q] = sum_i x_off[t,i] * (|x_off[t,i]| >= thresholds[i,n]) * W[i, n*Q+q]
                + out_mu[n*Q+q]

Sharding across 8 NeuronCores: 8-way tensor parallel over stripes (4 stripes
= 512 out cols per core); every core sees all 1024 tokens. The host passes
x pre-transposed (pure layout prep) so the device does no transposes at all,
and each core returns its y^T block which the host transposes back.

Per-core device algorithm:
  - x^T k-tiles [128 features, 1024 tokens] DMA'd contiguously (resident).
  - per (stripe n, k-tile): one custom DVE instruction computes
      z = select((x >= t_n) | (x <= -t_n), x, 0)
    which equals x * (|x| >= t_n) exactly in fp32 (single input stream).
  - PE matmul (float32r, N=512 moving) accumulating y^T[n-block] in PSUM
    over 32 k-tiles: acc += W[k,n].T @ z, two token-halves per stripe.
  - epilogue: ACT adds out_mu (per-partition bias in y^T layout), DMA out.
"""

import sys

if "/opt/trn_rl_repo" not in sys.path:
    sys.path.insert(0, "/opt/trn_rl_repo")

import numpy as np

import concourse.bass as bass
import concourse.mybir as mybir
import concourse.tile as tile
from concourse import bacc, bass_utils
from concourse import dve_ops as _dve_ops
from concourse.dve_spec import Spec, Src0, C0, C1, Zero, select, lower, PageIdx
from concourse.dve_spec import _has_src1
from concourse.dve_uop import DveOpSpec

# ---- problem constants -------------------------------------------------
B, S, I, N, Q = 2, 512, 4096, 32, 128
T = B * S                 # 1024 tokens
OUT = N * Q               # 4096
NCORES = 8
NS = N // NCORES          # 4 stripes per core
OUT_C = NS * Q            # 512 out cols per core
KT = I // 128             # 32 contraction tiles
P = 128
HF = 2                    # token halves per matmul group (1024 -> 2 x 512)
TH = T // HF              # 512

_CACHE = {}


def _gate2_ref(in0, in1, s0, s1, imm2):
    # in0: [P, S, N]; s0 = t(page0), s1 = dt with fl(t+dt) = t(page1) exactly
    steps = np.arange(in0.shape[1], dtype=np.float32)[None, :, None]
    t = (s0[:, :, None].astype(np.float32)
         + steps * s1[:, :, None].astype(np.float32)).astype(np.float32)
    return np.where((in0 >= t) | (in0 <= -t), in0, 0.0).astype(np.float32)


def _register_op(name, spec, subdim):
    if name in _dve_ops._SUB_OPCODE_FOR_NAME:
        return next(op for op in _dve_ops.OPS if op.name == name)
    row = max(_dve_ops._SUB_OPCODE_FOR_NAME.values()) + 1
    assert row < 0x20
    _dve_ops._SUB_OPCODE_FOR_NAME[name] = row
    shas = {}
    for ver in ("v3",):
        tmp = DveOpSpec(
            name=name, opcode=row, uops=lower(spec, ver=ver),
            rd1_en=_has_src1(spec),
        )
        shas[ver] = tmp.sha(ver)
    op = _dve_ops.DveOp(name, spec, subdim=subdim, uops_sha=shas)
    _dve_ops.OPS.append(op)
    _dve_ops.CUSTOM_DVE_SPECS[name] = spec
    return op


def _register_gate_op():
    """The fused CWIC gate for a PAIR of stripes as one custom DVE op.
    The per-page threshold is the affine value t_s = s0 + s*s1 (s in {0,1});
    the host picks s1 so fl(s0 + s1) equals the second stripe's threshold
    bit-exactly. out[p,s,:] = select((x >= t_s) | (x <= -t_s), x, 0)."""
    pg = PageIdx(C0, C1)
    spec = Spec(
        body=select((Src0 >= pg) | (Src0 <= Zero - pg), Src0, Zero),
        reference=_gate2_ref,
    )
    return _register_op("CWIC_GATE2_ANT", spec, subdim=True)


def _build():
    f32 = mybir.dt.float32
    f32r = mybir.dt.float32r
    gate_op = _register_gate_op()
    nc = bacc.Bacc("TRN2", target_bir_lowering=False, debug=False)

    xT_d = nc.dram_tensor("xT", [I, T], f32, kind="ExternalInput").ap()
    w_d = nc.dram_tensor("w", [I, OUT_C], f32, kind="ExternalInput").ap()
    # thr holds [t0, dt01, t2, dt23] per stripe pair (dt chosen so that
    # fl(t0+dt) reproduces the odd stripe's threshold bit-exactly)
    thr_d = nc.dram_tensor("thr", [I, NS], f32, kind="ExternalInput").ap()
    mu_d = nc.dram_tensor("mu", [P, NS], f32, kind="ExternalInput").ap()
    yT_d = nc.dram_tensor("yT", [OUT_C, T], f32, kind="ExternalOutput").ap()

    # w_v[p, k, c] = w[k*128+p, c]
    w_v = w_d.rearrange("(k p) c -> p k c", p=P)

    with tile.TileContext(nc) as tc:
        with (
            tc.tile_pool(name="const", bufs=1) as constp,
            tc.tile_pool(name="xT", bufs=20) as xTp,
            tc.tile_pool(name="thr", bufs=KT) as thrp,
            tc.tile_pool(name="w", bufs=12) as wp,
            tc.tile_pool(name="z", bufs=5) as zp,
            tc.tile_pool(name="yT", bufs=6) as yTp,
            tc.tile_pool(name="acc", bufs=2 * NS, space="PSUM") as accp,
        ):
            # interleave per-k threshold + x loads so z(n=0, k=0) can start
            # as soon as the first pair lands (per-tile dep granularity)
            xT = []
            thrT = []

            def load_pair(k):
                tk = thrp.tile([P, NS], f32, tag="thr", name=f"thr{k}")
                nc.sync.dma_start(tk[:], thr_d[k * P:(k + 1) * P, :])
                xk = xTp.tile([P, T], f32, tag="xT", name=f"xk{k}")
                nc.sync.dma_start(xk[:], xT_d[k * P:(k + 1) * P, :])
                xT.append(xk)
                thrT.append(tk)

            for k in range(2):
                load_pair(k)

            # HAM warm-up: a short burst of throwaway matmuls keyed on a DMA
            # that lands early in the x stream, so the PE clock is at 2.4 GHz
            # as real matmuls arrive (a cold PE runs at 1.2 GHz). The burst
            # writes into an accumulator bank that the real k=0 matmul
            # (start=True) resets afterwards.
            warmsrc = constp.tile([P, TH], f32r, tag="warmsrc")
            nc.sync.dma_start(warmsrc[:], xT_d[P:2 * P, 0:TH].bitcast(f32r))

            for k in range(2, KT):
                load_pair(k)

            # all W chunks issued upfront on the scalar engine's DMA queue;
            # transfers stream in consumption (r-major) order.
            # wc[p, kk*Q+q] = w[(r*KC+kk)*128+p, n*Q+q]
            KC = 8  # k-tiles per W chunk DMA
            wcs = {}
            for r in range(KT // KC):
                for n in range(NS):
                    wc = wp.tile([P, KC * Q], f32r, tag="w", name=f"wc{n}_{r}")
                    nc.scalar.dma_start(
                        wc[:].rearrange("p (k q) -> p k q", q=Q),
                        w_v[:, r * KC:(r + 1) * KC,
                            n * Q:(n + 1) * Q].bitcast(f32r),
                    )
                    wcs[(n, r)] = wc

            # out_mu is only needed at the epilogue — keep it off the
            # critical sync-queue head
            mu_sb = constp.tile([P, NS], f32, tag="mu")
            nc.scalar.dma_start(mu_sb[:], mu_d)

            accs = [
                accp.tile([P, TH], f32, tag="acc", name=f"acc{n}_{h}")
                for n in range(NS) for h in range(HF)
            ]
            for _ in range(6):
                nc.tensor.matmul(
                    accs[-1][:],
                    warmsrc[:, 0:P],
                    warmsrc[:],
                    start=True,
                    stop=True,
                )

            # k-outer: every x tile is consumed immediately by all 4 stripes
            # (as 2 stripe-pair gate ops), so the x stream spreads over the
            # whole kernel instead of piling into the first stripe's window.
            for k in range(KT):
                xk = xT[k][:]
                # x read twice via a zero-stride page dim: [P, 2, T]
                x_pg = bass.AP(xk.tensor, xk.offset,
                               [list(xk.ap[0]), [0, 2], list(xk.ap[1])])
                for pair in range(NS // 2):
                    zt = zp.tile([P, 2 * T], f32r, tag="z")
                    nc.vector._custom_dve(
                        gate_op,
                        out=zt[:].rearrange("p (s t) -> p s t", s=2),
                        in0=x_pg,
                        s0=thrT[k][:, 2 * pair:2 * pair + 1],
                        s1=thrT[k][:, 2 * pair + 1:2 * pair + 2],
                    )
                    for s in range(2):
                        n = 2 * pair + s
                        for h in range(HF):
                            nc.tensor.matmul(
                                accs[n * HF + h][:],
                                wcs[(n, k // KC)][:, (k % KC) * Q:
                                                  (k % KC + 1) * Q],
                                zt[:, s * T + h * TH:s * T + (h + 1) * TH],
                                start=(k == 0),
                                stop=(k == KT - 1),
                            )
            # epilogue: + out_mu (per-partition in y^T layout), DMA out
            for n in range(NS):
                for h in range(HF):
                    yt = yTp.tile([P, TH], f32, tag="yT")
                    nc.scalar.activation(
                        yt[:], accs[n * HF + h][:],
                        mybir.ActivationFunctionType.Identity,
                        bias=mu_sb[:, n:n + 1],
                    )
                    nc.sync.dma_start(
                        yT_d[n * P:(n + 1) * P, h * TH:(h + 1) * TH], yt[:]
                    )
    nc.compile()
    return nc


def _get_nc():
    if "nc" not in _CACHE:
        _CACHE["nc"] = _build()
    return _CACHE["nc"]


def _exact_dt(a, b):
    """fp32 dt with fl(a + dt) == b bit-exactly (monotone ulp adjustment)."""
    dt = (b - a).astype(np.float32)
    for _ in range(16):
        s = (a + dt).astype(np.float32)
        bad = s != b
        if not bad.any():
            return dt
        dt = np.where(bad & (s > b), np.nextafter(dt, np.float32(-np.inf)),
                      dt).astype(np.float32)
        dt = np.where(bad & (s < b), np.nextafter(dt, np.float32(np.inf)),
                      dt).astype(np.float32)
    raise AssertionError("exact stripe-pair threshold delta not reachable")


def _make_in_maps(x, W_kernel, thresholds, mu, out_mu):
    xf = np.asarray(x, dtype=np.float32).reshape(T, I)
    xf = xf - np.asarray(mu, dtype=np.float32)[None, :]
    xT = np.ascontiguousarray(xf.T)
    W = np.asarray(W_kernel, np.float32)
    thr = np.asarray(thresholds, np.float32)
    omu = np.asarray(out_mu, np.float32)
    in_maps = []
    for g in range(NCORES):
        thr_c = thr[:, g * NS:(g + 1) * NS]
        cols = []
        for pair in range(NS // 2):
            t0, t1 = thr_c[:, 2 * pair], thr_c[:, 2 * pair + 1]
            cols += [t0, _exact_dt(t0, t1)]
        in_maps.append({
            "xT": xT,
            "w": np.ascontiguousarray(W[:, g * OUT_C:(g + 1) * OUT_C]),
            "thr": np.ascontiguousarray(np.stack(cols, axis=1)),
            "mu": np.ascontiguousarray(
                omu[g * OUT_C:(g + 1) * OUT_C].reshape(NS, P).T
            ),
        })
    return in_maps


def _assemble(results):
    yT = np.concatenate([results[g]["yT"] for g in range(NCORES)], axis=0)
    return np.ascontiguousarray(yT.T).reshape(B, S, OUT)


def run(inputs, **spmd_kwargs):
    """Run on hardware; returns (y, BassKernelResults)."""
    nc = _get_nc()
    in_maps = _make_in_maps(
        inputs["x"], inputs["W_kernel"], inputs["thresholds"],
        inputs["mu"], inputs["out_mu"],
    )
    res = bass_utils.run_bass_kernel_spmd(
        nc, in_maps, core_ids=list(range(NCORES)), **spmd_kwargs
    )
    return _assemble(res.results), res


def kernel(x, W_kernel, thresholds, mu, out_mu, where):
    y, _ = run({
        "x": x, "W_kernel": W_kernel, "thresholds": thresholds,
        "mu": mu, "out_mu": out_mu, "where": where,
    })
    return y


# revision 35
# speedup vs baseline: 1.2721x; 1.0026x over previous
"""Trainium2 Bass kernel for the CWICDense (conditional stripe matmul) module.

Problem (hardcoded shapes):
  x          [2, 512, 4096] f32    tokens T=1024, features I=4096
  W_kernel   [4096, 4096]   f32    viewed as [I, N=32 stripes, Q=128]
  thresholds [4096, 32]     f32
  mu         [4096]         f32    (structurally zero in this module)
  out_mu     [4096]         f32
  where      [2, 512]       bool   (unused by the reference computation)

  y[t, n*Q+q] = sum_i x_off[t,i] * (|x_off[t,i]| >= thresholds[i,n]) * W[i, n*Q+q]
                + out_mu[n*Q+q]

Sharding across 8 NeuronCores: 8-way tensor parallel over stripes (4 stripes
= 512 out cols per core); every core sees all 1024 tokens. The host passes
x pre-transposed (pure layout prep) so the device does no transposes at all,
and each core returns its y^T block which the host transposes back.

Per-core device algorithm:
  - x^T k-tiles [128 features, 1024 tokens] DMA'd contiguously (resident).
  - per (stripe n, k-tile): one custom DVE instruction computes
      z = select((x >= t_n) | (x <= -t_n), x, 0)
    which equals x * (|x| >= t_n) exactly in fp32 (single input stream).
  - PE matmul (float32r, N=512 moving) accumulating y^T[n-block] in PSUM
    over 32 k-tiles: acc += W[k,n].T @ z, two token-halves per stripe.
  - epilogue: ACT adds out_mu (per-partition bias in y^T layout), DMA out.
"""

import sys

if "/opt/trn_rl_repo" not in sys.path:
    sys.path.insert(0, "/opt/trn_rl_repo")

import numpy as np

import concourse.bass as bass
import concourse.mybir as mybir
import concourse.tile as tile
from concourse import bacc, bass_utils
from concourse import dve_ops as _dve_ops
from concourse.dve_spec import Spec, Src0, C0, C1, Zero, select, lower, PageIdx
from concourse.dve_spec import _has_src1
from concourse.dve_uop import DveOpSpec

# ---- problem constants -------------------------------------------------
B, S, I, N, Q = 2, 512, 4096, 32, 128
T = B * S                 # 1024 tokens
OUT = N * Q               # 4096
NCORES = 8
NS = N // NCORES          # 4 stripes per core
OUT_C = NS * Q            # 512 out cols per core
KT = I // 128             # 32 contraction tiles
P = 128
HF = 2                    # token halves per matmul group (1024 -> 2 x 512)
TH = T // HF              # 512

_CACHE = {}


def _gate2_ref(in0, in1, s0, s1, imm2):
    # in0: [P, S, N]; s0 = t(page0), s1 = dt with fl(t+dt) = t(page1) exactly
    steps = np.arange(in0.shape[1], dtype=np.float32)[None, :, None]
    t = (s0[:, :, None].astype(np.float32)
         + steps * s1[:, :, None].astype(np.float32)).astype(np.float32)
    return np.where((in0 >= t) | (in0 <= -t), in0, 0.0).astype(np.float32)


def _register_op(name, spec, subdim):
    if name in _dve_ops._SUB_OPCODE_FOR_NAME:
        return next(op for op in _dve_ops.OPS if op.name == name)
    row = max(_dve_ops._SUB_OPCODE_FOR_NAME.values()) + 1
    assert row < 0x20
    _dve_ops._SUB_OPCODE_FOR_NAME[name] = row
    shas = {}
    for ver in ("v3",):
        tmp = DveOpSpec(
            name=name, opcode=row, uops=lower(spec, ver=ver),
            rd1_en=_has_src1(spec),
        )
        shas[ver] = tmp.sha(ver)
    op = _dve_ops.DveOp(name, spec, subdim=subdim, uops_sha=shas)
    _dve_ops.OPS.append(op)
    _dve_ops.CUSTOM_DVE_SPECS[name] = spec
    return op


def _register_gate_op():
    """The fused CWIC gate for a PAIR of stripes as one custom DVE op.
    The per-page threshold is the affine value t_s = s0 + s*s1 (s in {0,1});
    the host picks s1 so fl(s0 + s1) equals the second stripe's threshold
    bit-exactly. out[p,s,:] = select((x >= t_s) | (x <= -t_s), x, 0)."""
    pg = PageIdx(C0, C1)
    spec = Spec(
        body=select((Src0 >= pg) | (Src0 <= Zero - pg), Src0, Zero),
        reference=_gate2_ref,
    )
    return _register_op("CWIC_GATE2_ANT", spec, subdim=True)


def _build():
    f32 = mybir.dt.float32
    f32r = mybir.dt.float32r
    gate_op = _register_gate_op()
    nc = bacc.Bacc("TRN2", target_bir_lowering=False, debug=False)

    xT_d = nc.dram_tensor("xT", [I, T], f32, kind="ExternalInput").ap()
    w_d = nc.dram_tensor("w", [I, OUT_C], f32, kind="ExternalInput").ap()
    # thr holds [t0, dt01, t2, dt23] per stripe pair (dt chosen so that
    # fl(t0+dt) reproduces the odd stripe's threshold bit-exactly)
    thr_d = nc.dram_tensor("thr", [I, NS], f32, kind="ExternalInput").ap()
    mu_d = nc.dram_tensor("mu", [P, NS], f32, kind="ExternalInput").ap()
    yT_d = nc.dram_tensor("yT", [OUT_C, T], f32, kind="ExternalOutput").ap()

    # w_v[p, k, c] = w[k*128+p, c]
    w_v = w_d.rearrange("(k p) c -> p k c", p=P)

    with tile.TileContext(nc) as tc:
        with (
            tc.tile_pool(name="const", bufs=1) as constp,
            tc.tile_pool(name="xT", bufs=18) as xTp,
            tc.tile_pool(name="thr", bufs=KT) as thrp,
            tc.tile_pool(name="w", bufs=12) as wp,
            tc.tile_pool(name="z", bufs=5) as zp,
            tc.tile_pool(name="yT", bufs=8) as yTp,
            tc.tile_pool(name="acc", bufs=2 * NS, space="PSUM") as accp,
        ):
            # interleave per-k threshold + x loads so z(n=0, k=0) can start
            # as soon as the first pair lands (per-tile dep granularity)
            xT = []
            thrT = []

            def load_pair(k):
                tk = thrp.tile([P, NS], f32, tag="thr", name=f"thr{k}")
                nc.sync.dma_start(tk[:], thr_d[k * P:(k + 1) * P, :])
                xk = xTp.tile([P, T], f32, tag="xT", name=f"xk{k}")
                nc.sync.dma_start(xk[:], xT_d[k * P:(k + 1) * P, :])
                xT.append(xk)
                thrT.append(tk)

            for k in range(2):
                load_pair(k)

            # HAM warm-up: a short burst of throwaway matmuls keyed on a DMA
            # that lands early in the x stream, so the PE clock is at 2.4 GHz
            # as real matmuls arrive (a cold PE runs at 1.2 GHz). The burst
            # writes into an accumulator bank that the real k=0 matmul
            # (start=True) resets afterwards.
            warmsrc = constp.tile([P, TH], f32r, tag="warmsrc")
            nc.scalar.dma_start(warmsrc[:], xT_d[P:2 * P, 0:TH].bitcast(f32r))

            for k in range(2, KT):
                load_pair(k)

            # all W chunks issued upfront on the scalar engine's DMA queue;
            # transfers stream in consumption (r-major) order.
            # wc[p, kk*Q+q] = w[(r*KC+kk)*128+p, n*Q+q]
            KC = 8  # k-tiles per W chunk DMA
            wcs = {}
            for r in range(KT // KC):
                for n in range(NS):
                    wc = wp.tile([P, KC * Q], f32r, tag="w", name=f"wc{n}_{r}")
                    nc.scalar.dma_start(
                        wc[:].rearrange("p (k q) -> p k q", q=Q),
                        w_v[:, r * KC:(r + 1) * KC,
                            n * Q:(n + 1) * Q].bitcast(f32r),
                    )
                    wcs[(n, r)] = wc

            # out_mu is only needed at the epilogue — keep it off the
            # critical sync-queue head
            mu_sb = constp.tile([P, NS], f32, tag="mu")
            nc.scalar.dma_start(mu_sb[:], mu_d)

            accs = [
                accp.tile([P, TH], f32, tag="acc", name=f"acc{n}_{h}")
                for n in range(NS) for h in range(HF)
            ]
            for _ in range(6):
                nc.tensor.matmul(
                    accs[-1][:],
                    warmsrc[:, 0:P],
                    warmsrc[:],
                    start=True,
                    stop=True,
                )

            # k-outer: every x tile is consumed immediately by all 4 stripes
            # (as 2 stripe-pair gate ops), so the x stream spreads over the
            # whole kernel instead of piling into the first stripe's window.
            for k in range(KT):
                xk = xT[k][:]
                # x read twice via a zero-stride page dim: [P, 2, T]
                x_pg = bass.AP(xk.tensor, xk.offset,
                               [list(xk.ap[0]), [0, 2], list(xk.ap[1])])
                for pair in range(NS // 2):
                    zt = zp.tile([P, 2 * T], f32r, tag="z")
                    nc.vector._custom_dve(
                        gate_op,
                        out=zt[:].rearrange("p (s t) -> p s t", s=2),
                        in0=x_pg,
                        s0=thrT[k][:, 2 * pair:2 * pair + 1],
                        s1=thrT[k][:, 2 * pair + 1:2 * pair + 2],
                    )
                    for s in range(2):
                        n = 2 * pair + s
                        for h in range(HF):
                            nc.tensor.matmul(
                                accs[n * HF + h][:],
                                wcs[(n, k // KC)][:, (k % KC) * Q:
                                                  (k % KC + 1) * Q],
                                zt[:, s * T + h * TH:s * T + (h + 1) * TH],
                                start=(k == 0),
                                stop=(k == KT - 1),
                            )
            # epilogue: + out_mu (per-partition in y^T layout), DMA out.
            # Bias-adds alternate between ACT and the (idle by now) DVE so
            # the 8-tile tail chain halves in length.
            for n in range(NS):
                for h in range(HF):
                    yt = yTp.tile([P, TH], f32, tag="yT")
                    if (n * HF + h) % 2 == 0:
                        nc.scalar.activation(
                            yt[:], accs[n * HF + h][:],
                            mybir.ActivationFunctionType.Identity,
                            bias=mu_sb[:, n:n + 1],
                        )
                    else:
                        nc.vector.tensor_scalar(
                            yt[:], accs[n * HF + h][:], mu_sb[:, n:n + 1],
                            None, op0=mybir.AluOpType.add,
                        )
                    nc.sync.dma_start(
                        yT_d[n * P:(n + 1) * P, h * TH:(h + 1) * TH], yt[:]
                    )
    nc.compile()
    return nc


def _get_nc():
    if "nc" not in _CACHE:
        _CACHE["nc"] = _build()
    return _CACHE["nc"]


def _exact_dt(a, b):
    """fp32 dt with fl(a + dt) == b bit-exactly (monotone ulp adjustment)."""
    dt = (b - a).astype(np.float32)
    for _ in range(16):
        s = (a + dt).astype(np.float32)
        bad = s != b
        if not bad.any():
            return dt
        dt = np.where(bad & (s > b), np.nextafter(dt, np.float32(-np.inf)),
                      dt).astype(np.float32)
        dt = np.where(bad & (s < b), np.nextafter(dt, np.float32(np.inf)),
                      dt).astype(np.float32)
    raise AssertionError("exact stripe-pair threshold delta not reachable")


def _make_in_maps(x, W_kernel, thresholds, mu, out_mu):
    xf = np.asarray(x, dtype=np.float32).reshape(T, I)
    xf = xf - np.asarray(mu, dtype=np.float32)[None, :]
    xT = np.ascontiguousarray(xf.T)
    W = np.asarray(W_kernel, np.float32)
    thr = np.asarray(thresholds, np.float32)
    omu = np.asarray(out_mu, np.float32)
    in_maps = []
    for g in range(NCORES):
        thr_c = thr[:, g * NS:(g + 1) * NS]
        cols = []
        for pair in range(NS // 2):
            t0, t1 = thr_c[:, 2 * pair], thr_c[:, 2 * pair + 1]
            cols += [t0, _exact_dt(t0, t1)]
        in_maps.append({
            "xT": xT,
            "w": np.ascontiguousarray(W[:, g * OUT_C:(g + 1) * OUT_C]),
            "thr": np.ascontiguousarray(np.stack(cols, axis=1)),
            "mu": np.ascontiguousarray(
                omu[g * OUT_C:(g + 1) * OUT_C].reshape(NS, P).T
            ),
        })
    return in_maps


def _assemble(results):
    yT = np.concatenate([results[g]["yT"] for g in range(NCORES)], axis=0)
    return np.ascontiguousarray(yT.T).reshape(B, S, OUT)


def run(inputs, **spmd_kwargs):
    """Run on hardware; returns (y, BassKernelResults)."""
    nc = _get_nc()
    in_maps = _make_in_maps(
        inputs["x"], inputs["W_kernel"], inputs["thresholds"],
        inputs["mu"], inputs["out_mu"],
    )
    res = bass_utils.run_bass_kernel_spmd(
        nc, in_maps, core_ids=list(range(NCORES)), **spmd_kwargs
    )
    return _assemble(res.results), res


def kernel(x, W_kernel, thresholds, mu, out_mu, where):
    y, _ = run({
        "x": x, "W_kernel": W_kernel, "thresholds": thresholds,
        "mu": mu, "out_mu": out_mu, "where": where,
    })
    return y
